# revision 5
# baseline (speedup 1.0000x reference)
"""Trainium2 Bass kernel for nn_Dissection (layer-window QR pooling).

Math: the per-token QR of each layer-window reduces to the Cholesky factor
of the window's Gram submatrix (R^T R = G, align/novelty are sign-invariant),
so the kernel only needs the banded 9x9 per-token layer Gram (|i-j| <= 4).

Device pass A computes the 35 banded Gram entries per token (fused
multiply+accumulate on DVE). Host computes the tiny per-token Cholesky /
align / novelty / variance weights (16k x <=5x5, batched numpy). Device
pass B computes the weighted sum out[b] = sum_{t,l} W[b,t,l] * x[b,l,t,:]
on the tensor engine.

Sharding: pure data parallel over sentences, 16 per core on 8 cores.
"""
import numpy as np

import concourse.bass as bass
import concourse.tile as tile
from concourse import mybir
from concourse.bass_utils import run_bass_kernel_spmd

F32 = mybir.dt.float32

B, NL, S, D = 128, 13, 128, 768
LAYER_START = 4
L = NL - LAYER_START  # 9
T = S - 1  # 127
WINDOW = 2
N_CORES = 8
BPC = B // N_CORES  # 16 sentences per core

# banded gram pairs, diagonal-major: (i, i+d) for d in 0..4
PAIRS = [(i, i + d) for d in range(WINDOW * 2 + 1) for i in range(L - d)]
NPAIR = len(PAIRS)  # 35
PAIR_IDX = {p: idx for idx, p in enumerate(PAIRS)}
# start index of each diagonal band in PAIRS
BAND0 = [0, 9, 17, 24, 30]

DCH = 384  # free-dim chunk for PE matmuls (psum bank = 512 f32)
NCH = D // DCH  # 2


def _split_multiwait(nc, max_waits=1):
    """This walrus build encodes at most one sync-wait per instruction
    (compute and TPB_CTRL alike) and fails codegen with "Too many sync
    wait commands" otherwise. Split extras into single-wait NoOps placed
    immediately before the instruction on the same engine."""
    ctr = [0]

    def fresh():
        ctr[0] += 1
        return f"I-waitsplit-{ctr[0]}"

    for f in nc.m.functions:
        for bb in f.blocks:
            out = []
            for ins in bb.instructions:
                si = ins.sync_info
                if si is not None and si.on_wait and len(si.on_wait) > max_waits:
                    waits = list(si.on_wait)
                    for w in waits[:-max_waits]:
                        nop = mybir.InstNoOp(name=fresh(), ins=[], outs=[])
                        nop.engine = ins.engine
                        nop.sync_info = mybir.SyncInfo(on_wait=[w], on_update=[])
                        out.append(nop)
                    ins.sync_info = mybir.SyncInfo(
                        on_wait=waits[-max_waits:], on_update=list(si.on_update or [])
                    )
                out.append(ins)
            bb.instructions[:] = out
    return nc


def build_gram():
    """Pass A: banded per-token gram entries for BPC sentences."""
    nc = bass.Bass()
    xa = nc.declare_dram_parameter("xa", [BPC, NL, S, D], F32, isOutput=False)
    g = nc.declare_dram_parameter("g", [T, BPC * NPAIR], F32, isOutput=True)
    with tile.TileContext(nc) as tc:
        with (
            tc.tile_pool(name="xp", bufs=2) as xp,
            tc.tile_pool(name="gp", bufs=3) as gp,
            tc.tile_pool(name="scr", bufs=2) as scr,
        ):
            for b in range(BPC):
                xt = xp.tile([T, L, D], F32)
                src = xa[b, LAYER_START:NL, 0:T, :].rearrange("l t d -> t l d")
                nc.sync.dma_start(out=xt, in_=src)
                gt = gp.tile([T, NPAIR], F32)
                sc = scr.tile([T, D], F32)
                for idx, (i, j) in enumerate(PAIRS):
                    nc.vector.scalar_tensor_tensor(
                        out=sc,
                        in0=xt[:, i, :],
                        scalar=1.0,
                        in1=xt[:, j, :],
                        op0=mybir.AluOpType.mult,
                        op1=mybir.AluOpType.mult,
                        accum_out=gt[:, idx : idx + 1],
                    )
                nc.sync.dma_start(
                    out=g[:, b * NPAIR : (b + 1) * NPAIR], in_=gt
                )
    return _split_multiwait(nc)


def build_pool():
    """Pass B: out[b] = sum_{t,l} W[t, b, l] * x[b, l, t, :] on the PE."""
    nc = bass.Bass()
    xa = nc.declare_dram_parameter("xa", [BPC, NL, S, D], F32, isOutput=False)
    wp = nc.declare_dram_parameter("wp", [T, BPC * L], F32, isOutput=False)
    y = nc.declare_dram_parameter("y", [BPC, D], F32, isOutput=True)
    with tile.TileContext(nc) as tc:
        with (
            tc.tile_pool(name="xp", bufs=2) as xp,
            tc.tile_pool(name="wt", bufs=1) as wtp,
            tc.tile_pool(name="yrow", bufs=3) as yrow,
            tc.tile_pool(name="ps", bufs=4, space="PSUM") as ps,
        ):
            wt = wtp.tile([T, BPC * L], F32)
            nc.sync.dma_start(out=wt, in_=wp[:, :])
            for b in range(BPC):
                xt = xp.tile([T, L, D], F32)
                src = xa[b, LAYER_START:NL, 0:T, :].rearrange("l t d -> t l d")
                nc.sync.dma_start(out=xt, in_=src)
                yr = yrow.tile([1, D], F32)
                for c in range(NCH):
                    pt = ps.tile([1, DCH], F32)
                    for l in range(L):
                        nc.tensor.matmul(
                            out=pt,
                            lhsT=wt[:, b * L + l : b * L + l + 1],
                            rhs=xt[:, l, c * DCH : (c + 1) * DCH],
                            start=(l == 0),
                            stop=(l == L - 1),
                        )
                    nc.vector.tensor_copy(
                        out=yr[:, c * DCH : (c + 1) * DCH], in_=pt
                    )
                nc.sync.dma_start(out=y[b : b + 1, :], in_=yr)
    return _split_multiwait(nc)


def _rows_for_k(k):
    lo, hi = max(0, k - WINDOW), min(L, k + WINDOW + 1)
    return list(range(lo, k)) + list(range(k + 1, hi)) + [k]


class _WinMath:
    """Emits the per-window Cholesky + align/novelty math on [T, w, BPC]
    tiles (w = number of windows batched along the middle free axis)."""

    def __init__(self, nc, pool, width):
        self.nc, self.pool, self.w = nc, pool, width
        self.n = 0

    def tmp(self):
        self.n += 1
        return self.pool.tile([T, self.w, BPC], F32, tag=f"wm{self.w}_{self.n}")

    def mul(self, a, b):
        o = self.tmp()
        self.nc.vector.tensor_mul(out=o, in0=a, in1=b)
        return o

    def sub(self, a, b):
        o = self.tmp()
        self.nc.vector.tensor_sub(out=o, in0=a, in1=b)
        return o

    def add(self, a, b):
        o = self.tmp()
        self.nc.vector.tensor_add(out=o, in0=a, in1=b)
        return o

    def recip(self, a):
        o = self.tmp()
        self.nc.vector.reciprocal(out=o, in_=a)
        return o

    def sqrt(self, a):
        o = self.tmp()
        self.nc.scalar.activation(
            out=o, in_=a, func=mybir.ActivationFunctionType.Sqrt
        )
        return o

    def emit(self, m, a, s, sq0, s_center, aa_out, an_out):
        """a(i,j): gram AP (i>=j); s(j): rsqrt-diag AP for row j; sq0:
        sqrt-diag AP for row 0 (= l00); s_center: rsqrt center diag.
        Writes unnormalized aa ((m-1)/(2m) * nh / align_num) and nov."""
        nc = self.nc
        lmat = {(0, 0): sq0}
        r = {0: s(0)}
        d_last = None
        for j in range(m):
            # d_j = a(j,j) - sum_{k<j} l[j,k]^2
            cur = a(j, j)
            for k2 in range(j):
                cur = self.sub(cur, self.mul(lmat[(j, k2)], lmat[(j, k2)]))
            if j == m - 1:
                d_last = cur
                break
            if j > 0:
                ljj = self.sqrt(cur)
                lmat[(j, j)] = ljj
                r[j] = self.recip(ljj)
            for i in range(j + 1, m):
                cur2 = a(i, j)
                for k2 in range(j):
                    cur2 = self.sub(
                        cur2, self.mul(lmat[(i, k2)], lmat[(j, k2)])
                    )
                lmat[(i, j)] = self.mul(cur2, r[j])
        # mean_i = sum_{j=i..m-2} l[j,i]*s(j)   (mean_0 has +1 from l00*s0)
        align_num = None
        for i in range(m - 1):
            mean_i = None
            for j in range(max(i, 1), m - 1):
                term = self.mul(lmat[(j, i)], s(j))
                mean_i = term if mean_i is None else self.add(mean_i, term)
            if i == 0:
                if mean_i is None:
                    mean_i = self.tmp()
                    nc.vector.memset(mean_i, 1.0)
                else:
                    o = self.tmp()
                    nc.vector.tensor_scalar_add(out=o, in0=mean_i, scalar1=1.0)
                    mean_i = o
            term = self.mul(mean_i, lmat[(m - 1, i)])
            align_num = term if align_num is None else self.add(align_num, term)
        nh2 = self.sub(a(m - 1, m - 1), d_last)
        nh = self.sqrt(nh2)
        ra = self.recip(align_num)
        cm = (m - 1) / (2.0 * m)
        nc.vector.scalar_tensor_tensor(
            out=aa_out,
            in0=nh,
            scalar=cm,
            in1=ra,
            op0=mybir.AluOpType.mult,
            op1=mybir.AluOpType.mult,
        )
        dls = self.sqrt(d_last)
        nc.vector.tensor_mul(out=an_out, in0=dls, in1=s_center)


def build_fused():
    """Single-launch kernel: gram -> on-device weights -> weighted pool."""
    nc = bass.Bass()
    xa = nc.declare_dram_parameter("xa", [BPC, NL, S, D], F32, isOutput=False)
    y = nc.declare_dram_parameter("y", [BPC, D], F32, isOutput=True)
    vscr = nc.dram_tensor("vscr", [1, BPC], F32)
    with tile.TileContext(nc) as tc:
        with (
            tc.tile_pool(name="xp", bufs=2) as xp,
            tc.tile_pool(name="xq", bufs=2) as xq,
            tc.tile_pool(name="scr", bufs=2) as scrp,
            tc.tile_pool(name="gall", bufs=1) as gallp,
            tc.tile_pool(name="wtmp", bufs=1) as wtmp,
            tc.tile_pool(name="yrow", bufs=3) as yrow,
            tc.tile_pool(name="ps", bufs=4, space="PSUM") as ps,
            tc.tile_pool(name="psv", bufs=1, space="PSUM") as psv,
        ):
            # ---- phase A: banded gram entries for all sentences ----
            G = gallp.tile([T, NPAIR, BPC], F32)
            for b in range(BPC):
                xt = xp.tile([T, L, D], F32)
                src = xa[b, LAYER_START:NL, 0:T, :].rearrange("l t d -> t l d")
                nc.sync.dma_start(out=xt, in_=src)
                sc = scrp.tile([T, D], F32)
                for idx, (i, j) in enumerate(PAIRS):
                    nc.vector.scalar_tensor_tensor(
                        out=sc,
                        in0=xt[:, i, :],
                        scalar=1.0,
                        in1=xt[:, j, :],
                        op0=mybir.AluOpType.mult,
                        op1=mybir.AluOpType.mult,
                        accum_out=G[:, idx, b : b + 1],
                    )

            # ---- phase B: per-token weights, batched over all b ----
            # sqrt / rsqrt of the 9 diagonal entries
            SQ9 = wtmp.tile([T, L, BPC], F32)
            nc.scalar.activation(
                out=SQ9, in_=G[:, 0:L, :], func=mybir.ActivationFunctionType.Sqrt
            )
            S9 = wtmp.tile([T, L, BPC], F32)
            nc.vector.reciprocal(out=S9, in_=SQ9)

            AA = wtmp.tile([T, L, BPC], F32)
            AN = wtmp.tile([T, L, BPC], F32)

            # interior windows k=2..6 batched (m=5, rows [k-2,k-1,k+1,k+2,k])
            offs = [-2, -1, 1, 2, 0]
            wm5 = _WinMath(nc, wtmp, 5)

            def a5(i, j):
                ri, rj = offs[i], offs[j]
                lo, hi = min(ri, rj), max(ri, rj)
                idx = BAND0[hi - lo] + (2 + lo)
                return G[:, idx : idx + 5, :]

            def s5(j):
                return S9[:, 2 + offs[j] : 7 + offs[j], :]

            wm5.emit(
                5,
                a5,
                s5,
                SQ9[:, 0:5, :],
                S9[:, 2:7, :],
                AA[:, 2:7, :],
                AN[:, 2:7, :],
            )

            # boundary windows, one at a time
            for k in (0, 1, 7, 8):
                rows = _rows_for_k(k)
                m = len(rows)
                wm1 = _WinMath(nc, wtmp, 1)

                def a1(i, j, rows=rows):
                    ri, rj = rows[i], rows[j]
                    lo, hi = min(ri, rj), max(ri, rj)
                    idx = PAIR_IDX[(lo, hi)]
                    return G[:, idx : idx + 1, :]

                def s1(j, rows=rows):
                    return S9[:, rows[j] : rows[j] + 1, :]

                wm1.emit(
                    m,
                    a1,
                    s1,
                    SQ9[:, rows[0] : rows[0] + 1, :],
                    S9[:, k : k + 1, :],
                    AA[:, k : k + 1, :],
                    AN[:, k : k + 1, :],
                )

            # alpha = AA/sum(AA) + AN/sum(AN), normalized over layers
            saa = wtmp.tile([T, BPC], F32)
            nc.vector.tensor_reduce(
                out=saa,
                in_=AA.rearrange("p a b -> p b a"),
                axis=mybir.AxisListType.X,
                op=mybir.AluOpType.add,
            )
            san = wtmp.tile([T, BPC], F32)
            nc.vector.tensor_reduce(
                out=san,
                in_=AN.rearrange("p a b -> p b a"),
                axis=mybir.AxisListType.X,
                op=mybir.AluOpType.add,
            )
            rsaa = wtmp.tile([T, BPC], F32)
            nc.vector.reciprocal(out=rsaa, in_=saa)
            rsan = wtmp.tile([T, BPC], F32)
            nc.vector.reciprocal(out=rsan, in_=san)
            AL = wtmp.tile([T, L, BPC], F32)
            for l in range(L):
                t1 = wtmp.tile([T, BPC], F32, tag=f"al_t1_{l}")
                nc.vector.tensor_mul(out=t1, in0=AA[:, l, :], in1=rsaa)
                t2 = wtmp.tile([T, BPC], F32, tag=f"al_t2_{l}")
                nc.vector.tensor_mul(out=t2, in0=AN[:, l, :], in1=rsan)
                nc.vector.tensor_add(out=AL[:, l, :], in0=t1, in1=t2)
            sal = wtmp.tile([T, BPC], F32)
            nc.vector.tensor_reduce(
                out=sal,
                in_=AL.rearrange("p a b -> p b a"),
                axis=mybir.AxisListType.X,
                op=mybir.AluOpType.add,
            )
            rsal = wtmp.tile([T, BPC], F32)
            nc.vector.reciprocal(out=rsal, in_=sal)

            # var of adjacent-layer cosines (any global scale cancels)
            adj = wtmp.tile([T, L - 1, BPC], F32)
            nc.vector.tensor_mul(
                out=adj, in0=G[:, BAND0[1] : BAND0[1] + 8, :], in1=S9[:, 0:8, :]
            )
            nc.vector.tensor_mul(out=adj, in0=adj, in1=S9[:, 1:9, :])
            sadj = wtmp.tile([T, BPC], F32)
            nc.vector.tensor_reduce(
                out=sadj,
                in_=adj.rearrange("p a b -> p b a"),
                axis=mybir.AxisListType.X,
                op=mybir.AluOpType.add,
            )
            adj2 = wtmp.tile([T, L - 1, BPC], F32)
            nc.vector.tensor_mul(out=adj2, in0=adj, in1=adj)
            ssq = wtmp.tile([T, BPC], F32)
            nc.vector.tensor_reduce(
                out=ssq,
                in_=adj2.rearrange("p a b -> p b a"),
                axis=mybir.AxisListType.X,
                op=mybir.AluOpType.add,
            )
            sadj2 = wtmp.tile([T, BPC], F32)
            nc.vector.tensor_mul(out=sadj2, in0=sadj, in1=sadj)
            var_u = wtmp.tile([T, BPC], F32)
            # var*64 = 8*ssq - sadj^2  (scale cancels in var/sum(var))
            nc.vector.scalar_tensor_tensor(
                out=var_u,
                in0=ssq,
                scalar=8.0,
                in1=sadj2,
                op0=mybir.AluOpType.mult,
                op1=mybir.AluOpType.subtract,
            )

            # V[b] = sum_t var_u  -> broadcast 1/V to all partitions
            ones = wtmp.tile([T, 1], F32)
            nc.vector.memset(ones, 1.0)
            pv = psv.tile([1, BPC], F32)
            nc.tensor.matmul(out=pv, lhsT=ones, rhs=var_u, start=True, stop=True)
            vrow = wtmp.tile([1, BPC], F32)
            nc.vector.tensor_copy(out=vrow, in_=pv)
            nc.sync.dma_start(out=vscr[:, :], in_=vrow)
            vb = wtmp.tile([T, BPC], F32)
            nc.sync.dma_start(out=vb, in_=vscr[0:1, :].to_broadcast([T, BPC]))
            rv = wtmp.tile([T, BPC], F32)
            nc.vector.reciprocal(out=rv, in_=vb)

            # W[:, l, b] = AL * (rsal * var_u * rv)
            fac = wtmp.tile([T, BPC], F32)
            nc.vector.tensor_mul(out=fac, in0=rsal, in1=var_u)
            nc.vector.tensor_mul(out=fac, in0=fac, in1=rv)
            W = wtmp.tile([T, L, BPC], F32)
            for l in range(L):
                nc.vector.tensor_mul(out=W[:, l, :], in0=AL[:, l, :], in1=fac)

            # ---- phase C: weighted pooling on the PE ----
            for b in range(BPC):
                xt = xq.tile([T, L, D], F32)
                src = xa[b, LAYER_START:NL, 0:T, :].rearrange("l t d -> t l d")
                nc.sync.dma_start(out=xt, in_=src)
                yr = yrow.tile([1, D], F32)
                for c in range(NCH):
                    pt = ps.tile([1, DCH], F32)
                    for l in range(L):
                        nc.tensor.matmul(
                            out=pt,
                            lhsT=W[:, l, b : b + 1],
                            rhs=xt[:, l, c * DCH : (c + 1) * DCH],
                            start=(l == 0),
                            stop=(l == L - 1),
                        )
                    nc.vector.tensor_copy(
                        out=yr[:, c * DCH : (c + 1) * DCH], in_=pt
                    )
                nc.sync.dma_start(out=y[b : b + 1, :], in_=yr)
    return _split_multiwait(nc)


def weights_from_gram_entries(ent):
    """ent: [N, NPAIR] fp32 banded gram entries -> alpha [N, L], var [N]."""
    N = ent.shape[0]
    e = ent.astype(np.float64)
    G9 = np.zeros((N, L, L), np.float64)
    for idx, (i, j) in enumerate(PAIRS):
        G9[:, i, j] = e[:, idx]
        G9[:, j, i] = e[:, idx]
    aa = np.empty((N, L), np.float64)
    an = np.empty((N, L), np.float64)
    for k in range(L):
        rows = _rows_for_k(k)
        m = len(rows)
        Gw = G9[:, rows][:, :, rows]  # [N, m, m]
        Lc = np.linalg.cholesky(Gw)
        s = 1.0 / np.sqrt(np.einsum("nii->ni", Gw)[:, : m - 1])  # [N, m-1]
        mean = np.einsum("nji,nj->ni", Lc[:, : m - 1, : m - 1], s) / (m - 1)
        r_last = Lc[:, m - 1, :]  # [N, m]
        norm_head = np.linalg.norm(r_last[:, : m - 1], axis=1)
        align = np.einsum("ni,ni->n", mean, r_last[:, : m - 1]) / norm_head
        aa[:, k] = 1.0 / (align * m * 2)
        an[:, k] = r_last[:, m - 1] / np.sqrt(Gw[:, m - 1, m - 1])
    alpha = aa / aa.sum(1, keepdims=True) + an / an.sum(1, keepdims=True)
    alpha /= alpha.sum(1, keepdims=True)
    # adjacent-layer cosine variance from the d=1 band + diagonal
    diag = np.einsum("nii->ni", G9)
    adj = np.empty((N, L - 1), np.float64)
    for l in range(L - 1):
        adj[:, l] = G9[:, l, l + 1] / np.sqrt(diag[:, l] * diag[:, l + 1])
    var = adj.var(axis=1)
    return alpha, var


def kernel(all_layer_embeddings, masks):
    x = np.ascontiguousarray(np.asarray(all_layer_embeddings, dtype=np.float32))
    assert x.shape == (B, NL, S, D)
    core_ids = list(range(N_CORES))
    shards = [x[c * BPC : (c + 1) * BPC] for c in core_ids]

    # pass A: banded gram entries
    nc_a = build_gram()
    res_a = run_bass_kernel_spmd(
        nc_a, [{"xa": s} for s in shards], core_ids
    )
    # host: per-token weights
    wpacks = []
    for c in core_ids:
        g = res_a.results[c]["g"]  # [T, BPC*NPAIR]
        ent = (
            g.reshape(T, BPC, NPAIR).transpose(1, 0, 2).reshape(BPC * T, NPAIR)
        )
        alpha, var = weights_from_gram_entries(ent)
        alpha = alpha.reshape(BPC, T, L)
        var = var.reshape(BPC, T)
        w = var / var.sum(axis=1, keepdims=True)  # [BPC, T]
        W = (alpha * w[:, :, None]).astype(np.float32)  # [BPC, T, L]
        wpacks.append(np.ascontiguousarray(W.transpose(1, 0, 2).reshape(T, BPC * L)))

    # pass B: weighted pooling
    nc_b = build_pool()
    res_b = run_bass_kernel_spmd(
        nc_b,
        [{"xa": s, "wp": wp} for s, wp in zip(shards, wpacks)],
        core_ids,
    )
    out = np.concatenate([res_b.results[c]["y"] for c in core_ids], axis=0)
    return out.astype(np.float32)
